# revision 33
# baseline (speedup 1.0000x reference)
"""GRU (BubblePredictor) Trainium2 Bass kernel.

Problem: B=128, T=128, I=12, H=512 GRU (PyTorch gate order r,z,n, no bias)
with a per-step 2-wide prediction head.

Strategy: data-parallel over batch across 8 NeuronCores (16 examples/core),
no collectives. Per core, the sequential recurrence keeps the hidden state
batch-transposed (h.T, [H on partitions x batch free], fp16) as the matmul
stationary operand and W_hh.T (fp16) as the moving operand, with PE
column-group tiling running four matmul chains concurrently: the r gate is
split into two 256-wide halves (col-groups 0 and 3) so sigmoid(r) starts
early, h_n on cg1, z on cg2. Input-side preacts for r/z are folded into the
same PSUM accumulation as a K=12 round with x_t.T stationary. The n-gate
tail runs in transposed space: t1 = sigmoid(r) * h_n is PE-transposed per
128-chunk with i_n.T accumulated on top via a K=12 matmul, so tanh and the
(1-z)*n + z*h blend are cheap FD=64 ops, and h_new.T is written straight
back to the fp16 h.T history buffer for the next step's stationary operand.
The per-step prediction head is deferred: all h_t.T stay in SBUF and one
batched matmul against W_pred.T at the end produces every logit.
"""

import numpy as np
from contextlib import ExitStack

B, T, I, H = 128, 128, 12, 512
G = 3 * H
NCORES = 8
BL = B // NCORES  # 16 examples per core

_cache = {}


def _build():
    import concourse.bacc as bacc
    import concourse.mybir as mybir
    import concourse.tile as tile
    from concourse.masks import make_identity

    F32 = mybir.dt.float32
    F16 = mybir.dt.float16
    AF = mybir.ActivationFunctionType
    OP = mybir.AluOpType

    nc = bacc.Bacc("TRN2", target_bir_lowering=False, debug=False, num_devices=1)

    d_histT = nc.dram_tensor("histT", [I, T * BL], F16, kind="ExternalInput")
    d_whhT = nc.dram_tensor("whhT", [H, G], F16, kind="ExternalInput")
    d_wihT = nc.dram_tensor("wihT", [I, G], F16, kind="ExternalInput")
    d_wpredT = nc.dram_tensor("wpredT", [H, 2], F16, kind="ExternalInput")
    d_logits = nc.dram_tensor("logitsT", [2, T, BL], F32, kind="ExternalOutput")
    d_hlast = nc.dram_tensor("hlastT", [4, 128, BL], F16, kind="ExternalOutput")

    with tile.TileContext(nc) as tc, ExitStack() as ctx:
        const = ctx.enter_context(tc.tile_pool(name="const", bufs=1))
        sb = ctx.enter_context(tc.tile_pool(name="sb", bufs=4))
        pg_pool = ctx.enter_context(tc.tile_pool(name="pg", bufs=2, space="PSUM"))
        pq_pool = ctx.enter_context(tc.tile_pool(name="pq", bufs=2, space="PSUM"))
        pz_pool = ctx.enter_context(tc.tile_pool(name="pz", bufs=2, space="PSUM"))
        pl_pool = ctx.enter_context(tc.tile_pool(name="pl", bufs=2, space="PSUM"))

        ident = const.tile([128, 128], F32)
        make_identity(nc, ident[:])

        whh = const.tile([128, 4, G], F16)
        nc.sync.dma_start(whh[:], d_whhT.ap().rearrange("(k p) g -> p k g", p=128))
        wih = const.tile([I, G], F16)
        nc.sync.dma_start(wih[:], d_wihT.ap())
        histT = const.tile([I, T * BL], F16)
        nc.sync.dma_start(histT[:], d_histT.ap())
        wpred = const.tile([128, 4, 2], F16)
        nc.sync.dma_start(wpred[:], d_wpredT.ap().rearrange("(k p) d -> p k d", p=128))

        # h_t.T history (f16, matmul operand): [128, t, k, b], H index = k*128+p.
        # Slot t holds the hidden state *entering* step t (slot 0 = h0 = 0).
        hT = const.tile([128, T + 1, 4, BL], F16)
        nc.vector.memset(hT[:, 0, :, :].bitcast(F32), 0.0)

        for t in range(T):
            psum_g = pg_pool.tile([128, 512], F32)
            xt = histT[:, t * BL:(t + 1) * BL]

            # gate preacts. r is split into two 256-wide halves on col-groups
            # 0 (parts 0:16, cols 0:256) and 3 (parts 96:112, cols 256:512) so
            # both halves' matmul chains run concurrently and sigmoid(r) can
            # start ~2x earlier. h_n -> cg1 (32:48), z -> cg2 (64:80). r/z get
            # their input-side preact folded in as a K=12 round with x_t.T
            # stationary (issued first so it can run before h_t is ready).
            for half, p0 in ((0, 0), (1, 96)):
                cols = slice(256 * half, 256 * (half + 1))
                nc.tensor.matmul(
                    psum_g[p0:p0 + BL, cols],
                    lhsT=xt,
                    rhs=wih[:, cols],
                    start=True,
                    stop=False,
                    tile_position=(0, p0),
                )
                for k in range(4):
                    nc.tensor.matmul(
                        psum_g[p0:p0 + BL, cols],
                        lhsT=hT[:, t, k, :],
                        rhs=whh[:, k, cols],
                        start=False,
                        stop=(k == 3),
                        tile_position=(0, p0),
                    )
            for cg, g in ((1, 2), (2, 1)):
                p0 = 32 * cg
                if g != 2:
                    nc.tensor.matmul(
                        psum_g[p0:p0 + BL, :],
                        lhsT=xt,
                        rhs=wih[:, 512 * g:512 * (g + 1)],
                        start=True,
                        stop=False,
                        tile_position=(0, p0),
                    )
                for k in range(4):
                    nc.tensor.matmul(
                        psum_g[p0:p0 + BL, :],
                        lhsT=hT[:, t, k, :],
                        rhs=whh[:, k, 512 * g:512 * (g + 1)],
                        start=(k == 0 and g == 2),
                        stop=(k == 3),
                        tile_position=(0, p0),
                    )

            r_sb = sb.tile([BL, 512], F32)
            nc.scalar.activation(r_sb[:, 0:256], psum_g[0:BL, 0:256], AF.Sigmoid)
            nc.scalar.activation(r_sb[:, 256:512], psum_g[96:96 + BL, 256:512], AF.Sigmoid)
            z_sb = sb.tile([BL, 512], F32)
            nc.scalar.activation(z_sb[:], psum_g[64:64 + BL, :], AF.Sigmoid)

            t1 = sb.tile([BL, 512], F32)
            nc.vector.tensor_tensor(
                t1[:, 0:256], r_sb[:, 0:256], psum_g[32:48, 0:256], OP.mult
            )
            nc.vector.tensor_tensor(
                t1[:, 256:512], r_sb[:, 256:512], psum_g[32:48, 256:512], OP.mult
            )

            # q.T = transpose(t1) + i_n.T, accumulated on the PE per 128-chunk:
            # PE-transpose of t1's chunk opens the group (transpose-mode writes
            # overwrite, so it must come first), then a K=12 matmul adds i_n.T.
            psum_q = pq_pool.tile([128, 4, BL], F32)
            for k in range(4):
                nc.tensor.transpose(
                    psum_q[:, k, :],
                    t1[:, 128 * k:128 * (k + 1)],
                    ident[0:BL, 0:BL],
                )
                nc.tensor.matmul(
                    psum_q[:, k, :],
                    lhsT=wih[:, 1024 + 128 * k:1024 + 128 * (k + 1)],
                    rhs=xt,
                    start=False,
                    stop=True,
                    skip_group_check=True,
                )
            nT = sb.tile([128, 4 * BL], F32)
            nc.scalar.activation(
                nT[:], psum_q[:].rearrange("p k b -> p (k b)"), AF.Tanh
            )

            # z.T via PE transpose of z; z'T = 1 - z.T on DVE.
            psum_zT = pz_pool.tile([128, 4, BL], F32)
            for k in range(4):
                nc.tensor.transpose(
                    psum_zT[:, k, :],
                    z_sb[:, 128 * k:128 * (k + 1)],
                    ident[0:BL, 0:BL],
                )
            zT = psum_zT[:].rearrange("p k b -> p (k b)")

            # h_new.T = (1-z.T)*n.T + z.T*h.T -- only the n.T product and the
            # final add sit after tanh on the critical chain; z' and z*h are
            # computed as soon as z.T lands.
            zpT = sb.tile([128, 4 * BL], F32)
            nc.vector.tensor_scalar(zpT[:], zT, -1.0, 1.0, OP.mult, OP.add)
            bT = sb.tile([128, 4 * BL], F32)
            nc.vector.tensor_tensor(
                bT[:], zT, hT[:, t, :, :].rearrange("p k b -> p (k b)"), OP.mult
            )
            aT = sb.tile([128, 4 * BL], F32)
            nc.vector.tensor_tensor(aT[:], zpT[:], nT[:], OP.mult)
            nc.vector.tensor_tensor(
                hT[:, t + 1, :, :].rearrange("p k b -> p (k b)"),
                aT[:], bT[:], OP.add,
            )

        # prediction head: logits.T [2, T*BL] = wpred.T @ hT[1:]
        logT = const.tile([2, T * BL], F32)
        for c in range(4):
            psum_l = pl_pool.tile([2, 512], F32)
            for k in range(4):
                nc.tensor.matmul(
                    psum_l[:, :],
                    lhsT=wpred[:, k, :],
                    rhs=hT[:, 1 + 32 * c:1 + 32 * (c + 1), k, :],
                    start=(k == 0),
                    stop=(k == 3),
                )
            nc.vector.tensor_copy(logT[:, 512 * c:512 * (c + 1)], psum_l[:, :])

        nc.sync.dma_start(
            d_logits.ap().rearrange("d t b -> d (t b)"),
            logT[:],
        )
        for k in range(4):
            nc.sync.dma_start(d_hlast.ap()[k, :, :], hT[:, T, k, :])

    nc.compile()
    return nc


def _get_nc():
    if "nc" not in _cache:
        _cache["nc"] = _build()
    return _cache["nc"]


def kernel(history, W_ih, W_hh, W_pred):
    from concourse.bass_utils import run_bass_kernel_spmd

    nc = _get_nc()
    in_maps = make_in_maps(
        {"history": history, "W_ih": W_ih, "W_hh": W_hh, "W_pred": W_pred}
    )
    res = run_bass_kernel_spmd(nc, in_maps, core_ids=list(range(NCORES)))
    return _assemble(res)


def _assemble(res):
    logits = np.concatenate(
        [r["logitsT"].transpose(2, 1, 0) for r in res.results], axis=0
    )
    h_last = np.concatenate(
        [
            r["hlastT"].transpose(2, 0, 1).reshape(BL, H).astype(np.float32)
            for r in res.results
        ],
        axis=0,
    )
    return logits, h_last


def make_in_maps(inputs):
    history = np.asarray(inputs["history"], dtype=np.float32)
    whhT = np.ascontiguousarray(np.asarray(inputs["W_hh"], np.float32).T.astype(np.float16))
    wihT = np.ascontiguousarray(np.asarray(inputs["W_ih"], np.float32).T.astype(np.float16))
    wpredT = np.ascontiguousarray(np.asarray(inputs["W_pred"], np.float32).T.astype(np.float16))
    in_maps = []
    for c in range(NCORES):
        hc = history[c * BL:(c + 1) * BL]
        histT = np.ascontiguousarray(hc.transpose(2, 1, 0).reshape(I, T * BL).astype(np.float16))
        in_maps.append(
            {"histT": histT, "whhT": whhT, "wihT": wihT, "wpredT": wpredT}
        )
    return in_maps


def run_traced(inputs):
    from concourse.bass_utils import run_bass_kernel_spmd

    nc = _get_nc()
    return run_bass_kernel_spmd(
        nc, make_in_maps(inputs), core_ids=list(range(NCORES)), trace=True
    )
